# revision 1
# baseline (speedup 1.0000x reference)
"""GCN layer (degree-normalized SpMM + dense matmul) on 8 Trainium2 cores.

out = D^-1/2 A D^-1/2 feat W + b, A built from 600K (src, dst) edges.

Sharding: destination nodes across 8 cores (12500 each). Within a core,
nodes are greedily re-packed into 98 windows of 128 so that each
(window, src-bank) pair holds <= 256 incoming edges (the measured max is
~196 vs mean 191). feat is replicated per core, split into 4 row-banks
of 25000 (int16 index range for the custom gather ucode).

Device pipeline per window:
  - 4x dma_gather (one per bank, one SWDGE queue each -> parallel Q7
    descriptor generation) pull the window's source feature rows into
    SBUF as 8 chunks of [128 edges, 128 feat].
  - Per chunk, the vector engine builds onehot[e, v] =
    (iota[v] == dst_slot[e]) * norm_src[e] in one tensor_scalar op.
  - TensorE accumulates agg^T[din, v] += X_chunk^T @ onehot in PSUM.
  - TensorE computes (agg^T)^T @ W -> [v, dout] in PSUM; the scalar
    engine applies the per-node norm[dst] scale on the PSUM->SBUF copy;
    the vector engine adds the (broadcast) bias; HWDGE writes the
    window's 128 output rows.

Host-side work is shard construction only: degree histograms + rsqrt
norms, node re-packing, edge bucketing/padding, constant tables, and
the inverse node permutation at unshard.
"""

import numpy as np

N_NODES = 100000
N_EDGES = 600000
D = 128
NC = 8            # cores
NPC = 12500       # nodes per core
P = 128           # partitions / window size
W = 98            # windows per core
NB = 4            # feat banks
BS = 25000        # bank size (int16-addressable)
CAP = 256         # max edges per (window, bank) -> 2 chunks of 128
CPW = NB * 2      # chunks per window
XG_BUFS = 16


def _build_bass(caps, rep=None, parts="all", bufs=None):
    """caps: [W, NB] int array of static per-(window, bank) index counts
    (128 < caps <= 256, shared across cores). rep: wrap the window loop
    in a hardware For_i for benchmarking. parts: 'all' | 'gather' |
    'compute' | 'onehot' | 'matmul' to isolate stages when benchmarking."""
    import concourse.bacc as bacc
    import concourse.bass as bass
    import concourse.mybir as mybir
    import concourse.tile as tile

    f32 = mybir.dt.float32
    i16 = mybir.dt.int16

    # idx column layout: per (w, b) a block of ceil(cap/16) int16 columns,
    # rounded to even so every block starts 4B-aligned
    ncols = [(-(-int(caps[w, b]) // 16) + 1) // 2 * 2 for w in range(W) for b in range(NB)]
    col0 = np.concatenate([[0], np.cumsum(ncols)])
    tot_cols = int(col0[-1])

    do_gather = parts in ("all", "gather")
    do_onehot = parts in ("all", "compute", "onehot")
    do_matmul = parts in ("all", "compute", "matmul")
    do_tail = parts in ("all", "compute")
    XB = bufs or XG_BUFS

    nc = bacc.Bacc(
        None,
        target_bir_lowering=False,
        dynamic_dma_scratch_size=32768,
        num_swdge_queues=4,
    )
    feat_b = [
        nc.declare_dram_parameter(f"feat{b}", [BS, D], f32, isOutput=False)
        for b in range(NB)
    ]
    w_d = nc.declare_dram_parameter("w", [D, D], f32, isOutput=False)
    biasb_d = nc.declare_dram_parameter("biasb", [P, D], f32, isOutput=False)
    iota_d = nc.declare_dram_parameter("iota", [P, P], f32, isOutput=False)
    idx_d = nc.declare_dram_parameter("idx", [P, tot_cols], i16, isOutput=False)
    dstc_d = nc.declare_dram_parameter("dstc", [P, W * CPW], f32, isOutput=False)
    sed_d = nc.declare_dram_parameter("sed", [P, W * CPW], f32, isOutput=False)
    normd_d = nc.declare_dram_parameter("normd", [P, W], f32, isOutput=False)
    out_d = nc.declare_dram_parameter("out", [W * P, D], f32, isOutput=True)

    with tile.TileContext(nc) as tc:
        with (
            tc.tile_pool(name="const", bufs=1) as cp,
            tc.tile_pool(name="xg", bufs=XB) as xp,
            tc.tile_pool(name="oh", bufs=8) as ohp,
            tc.tile_pool(name="sb", bufs=8) as sbp,
            tc.tile_pool(name="osb", bufs=8) as obp,
            tc.tile_pool(name="ps1", bufs=6, space="PSUM") as pp1,
            tc.tile_pool(name="ps2", bufs=2, space="PSUM") as pp2,
        ):
            idx_sb = cp.tile([P, tot_cols], i16)
            nc.sync.dma_start(out=idx_sb[:], in_=idx_d[:])
            dstc_sb = cp.tile([P, W * CPW], f32)
            nc.sync.dma_start(out=dstc_sb[:], in_=dstc_d[:])
            sed_sb = cp.tile([P, W * CPW], f32)
            nc.sync.dma_start(out=sed_sb[:], in_=sed_d[:])
            normd_sb = cp.tile([P, W], f32)
            nc.sync.dma_start(out=normd_sb[:], in_=normd_d[:])
            iota_sb = cp.tile([P, P], f32)
            nc.sync.dma_start(out=iota_sb[:], in_=iota_d[:])
            biasb_sb = cp.tile([P, D], f32)
            nc.sync.dma_start(out=biasb_sb[:], in_=biasb_d[:])
            w_sb = cp.tile([D, D], f32)
            nc.sync.dma_start(out=w_sb[:], in_=w_d[:])

            import contextlib

            loop_cm = tc.For_i(0, rep, 1) if rep else contextlib.nullcontext()
            with loop_cm:
                for w_i in range(W):
                    xg = xp.tile([P, CPW * D], f32, tag="xg")
                    if rep is None and w_i < XB:
                        # first use of each buf slot: clear so skipped
                        # trailing slots hold finite data (NaN safety)
                        nc.vector.memset(xg[:], 0.0)
                    if not do_gather and do_matmul:
                        # benchmark mode: xg needs a writer (Pool engine,
                        # off the critical DVE/PE path)
                        nc.gpsimd.memset(xg[:], 0.0)
                    for b in (range(NB) if do_gather else []):
                        g = w_i * NB + b
                        n = int(caps[w_i, b])
                        nic = -(-n // 16)
                        nc.gpsimd.dma_gather(
                            out_ap=xg[:, b * 2 * D : (b + 1) * 2 * D].rearrange(
                                "p (c r) -> p c r", r=D
                            ),
                            in_ap=feat_b[b][:, :],
                            idxs_ap=idx_sb[:, int(col0[g]) : int(col0[g]) + nic],
                            num_idxs=n,
                            num_idxs_reg=n,
                            elem_size=D,
                            queue_num=b,
                        )
                    psA = pp1.tile([P, P], f32, tag="psA")
                    for j in (range(CPW) if (do_onehot or do_matmul) else []):
                        col = w_i * CPW + j
                        oh = ohp.tile([P, P], f32, tag="oh")
                        if do_onehot:
                            nc.vector.tensor_scalar(
                            out=oh[:],
                            in0=iota_sb[:],
                            scalar1=dstc_sb[:, col : col + 1],
                            scalar2=sed_sb[:, col : col + 1],
                                op0=mybir.AluOpType.is_equal,
                                op1=mybir.AluOpType.mult,
                            )
                        if do_matmul:
                            nc.tensor.matmul(
                                out=psA[:],
                                lhsT=xg[:, j * D : (j + 1) * D],
                                rhs=oh[:],
                                start=(j == 0),
                                stop=(j == CPW - 1),
                            )
                    if not do_tail:
                        continue
                    aggT = sbp.tile([P, P], f32, tag="aggT")
                    nc.scalar.activation(
                        aggT[:], psA[:], mybir.ActivationFunctionType.Copy
                    )
                    psB = pp2.tile([P, D], f32, tag="psB")
                    nc.tensor.matmul(
                        out=psB[:], lhsT=aggT[:], rhs=w_sb[:], start=True, stop=True
                    )
                    hsb = obp.tile([P, D], f32, tag="hsb")
                    nc.scalar.activation(
                        hsb[:],
                        psB[:],
                        mybir.ActivationFunctionType.Copy,
                        scale=normd_sb[:, w_i : w_i + 1],
                    )
                    osb = obp.tile([P, D], f32, tag="osb")
                    nc.vector.tensor_add(out=osb[:], in0=hsb[:], in1=biasb_sb[:])
                    nc.sync.dma_start(
                        out=out_d[w_i * P : (w_i + 1) * P, :], in_=osb[:]
                    )
    nc.compile()
    return nc


def _prep_shards(feat, weight, bias, src, dst):
    feat = np.ascontiguousarray(np.asarray(feat, dtype=np.float32))
    weight = np.ascontiguousarray(np.asarray(weight, dtype=np.float32))
    bias = np.asarray(bias, dtype=np.float32)
    src = np.asarray(src, dtype=np.int64)
    dst = np.asarray(dst, dtype=np.int64)

    deg = np.bincount(dst, minlength=N_NODES)
    norm = (1.0 / np.sqrt(np.maximum(deg, 1.0))).astype(np.float32)
    bank = src // BS

    # per-node per-bank in-degree, for window packing
    d4 = np.zeros((N_NODES, NB), np.int64)
    for b in range(NB):
        np.add.at(d4[:, b], dst[bank == b], 1)

    # greedy re-pack of each core's nodes into W windows of <=128 nodes,
    # balancing the per-bank edge loads
    slot_of = np.full(N_NODES, -1, np.int32)   # node -> slot (0..127)
    win_of = np.full(N_NODES, -1, np.int32)    # node -> window (0..97)
    perm = np.full((NC, W * P), -1, np.int64)  # (core, w*128+p) -> node
    for m in range(NC):
        nodes = np.arange(m * NPC, (m + 1) * NPC)
        dv = d4[nodes]
        order = np.argsort(-dv.sum(1), kind="stable")
        loads = np.zeros((W, NB), np.int64)
        counts = np.zeros(W, np.int32)
        for i in order:
            cand = (loads + dv[i]).max(1)
            cand[counts >= P] = 1 << 40
            w = int(np.argmin(cand))
            n = nodes[i]
            win_of[n] = w
            slot_of[n] = counts[w]
            perm[m, w * P + counts[w]] = n
            loads[w] += dv[i]
            counts[w] += 1

    # bucket edges by (core, window, bank); position within bucket
    core_e = dst // NPC
    w_e = win_of[dst]
    key = (core_e * W + w_e) * NB + bank
    order = np.argsort(key, kind="stable")
    srcs, dsts, keys = src[order], dst[order], key[order]
    counts_e = np.bincount(keys, minlength=NC * W * NB)
    starts = np.zeros(NC * W * NB, np.int64)
    np.cumsum(counts_e[:-1], out=starts[1:])
    within = np.arange(N_EDGES, dtype=np.int64) - starts[keys]

    cnt3 = counts_e.reshape(NC, W, NB)
    caps = cnt3.max(axis=0)  # [W, NB] static counts shared by all cores
    assert caps.max() <= CAP, f"window/bank overflow: {caps.max()}"
    assert caps.min() > P, f"cap {caps.min()} <= 128 breaks uniform 2-chunk shape"

    # slot-dense arrays [NC, W, NB, CAP]
    idx_full = np.zeros((NC, W, NB, CAP), np.int16)
    dstc_full = np.full((NC, W, NB, CAP), 255.0, np.float32)
    sed_full = np.zeros((NC, W, NB, CAP), np.float32)
    flat = ((keys * CAP) + within).astype(np.int64)
    idx_full.reshape(-1)[flat] = (srcs % BS).astype(np.int16)
    dstc_full.reshape(-1)[flat] = slot_of[dsts]
    sed_full.reshape(-1)[flat] = norm[srcs]

    # gather idx layout: per (w,b) block of ceil(cap/16) cols rounded even,
    # value i at [i % 16, block + i // 16], replicated across 8 core-groups
    ncols = [(-(-int(caps[w, b]) // 16) + 1) // 2 * 2 for w in range(W) for b in range(NB)]
    col0 = np.concatenate([[0], np.cumsum(ncols)]).astype(np.int64)
    tot_cols = int(col0[-1])
    idx_dev = np.zeros((NC, 16, tot_cols), np.int16)
    for w in range(W):
        for b in range(NB):
            g = w * NB + b
            n = int(caps[w, b])
            blk = idx_full[:, w, b, : -(-n // 16) * 16].reshape(NC, -1, 16)
            idx_dev[:, :, col0[g] : col0[g] + blk.shape[1]] = blk.transpose(0, 2, 1)
    idx_dev = np.tile(idx_dev, (1, 8, 1))  # replicate to 128 partitions

    # onehot metadata [NC, 128, W*CPW]: chunk j of (w,b) -> column w*8+b*2+j
    dstc_dev = (
        dstc_full.reshape(NC, W, NB * 2, P).transpose(0, 3, 1, 2).reshape(NC, P, W * CPW)
    )
    sed_dev = (
        sed_full.reshape(NC, W, NB * 2, P).transpose(0, 3, 1, 2).reshape(NC, P, W * CPW)
    )

    norm_perm = np.where(perm >= 0, norm[np.maximum(perm, 0)], 0.0).astype(np.float32)
    normd = norm_perm.reshape(NC, W, P).transpose(0, 2, 1)  # [NC, 128, W]

    iota = np.broadcast_to(np.arange(P, dtype=np.float32), (P, P)).copy()
    biasb = np.broadcast_to(bias, (P, D)).copy()
    banks = [np.ascontiguousarray(feat[b * BS : (b + 1) * BS]) for b in range(NB)]

    in_maps = []
    for m in range(NC):
        im = {f"feat{b}": banks[b] for b in range(NB)}
        im.update(
            w=weight,
            biasb=biasb,
            iota=iota,
            idx=np.ascontiguousarray(idx_dev[m]),
            dstc=np.ascontiguousarray(dstc_dev[m]),
            sed=np.ascontiguousarray(sed_dev[m]),
            normd=np.ascontiguousarray(normd[m]),
        )
        in_maps.append(im)
    return in_maps, caps, perm


def kernel(feat, weight, bias, src, dst):
    from concourse.bass_utils import run_bass_kernel_spmd

    in_maps, caps, perm = _prep_shards(feat, weight, bias, src, dst)
    nc = _build_bass(caps)
    res = run_bass_kernel_spmd(nc, in_maps, list(range(NC)))
    out = np.empty((N_NODES, D), np.float32)
    for m in range(NC):
        o = res.results[m]["out"]
        mask = perm[m] >= 0
        out[perm[m][mask]] = o[mask]
    return out



# revision 57
# speedup vs baseline: 2.8691x; 2.8691x over previous
"""GCN layer (degree-normalized SpMM + dense matmul) on 8 Trainium2 cores.

out = D^-1/2 A D^-1/2 feat W + b, A built from 600K (src, dst) edges.

Sharding: destination nodes across 8 cores (12500 each). Within a core,
nodes are greedily re-packed into 98 windows of 128 against rotating
per-bank capacity targets: light banks (128 slots) rotate across
windows — two light banks on the first W2=46 windows, one on the rest
(6-7 edge-chunks of 128 per window, ~82k gather descriptors/core);
the packer cascades to fewer light banks if packing overflows. feat
rows are
pre-scaled by norm[src] on the host, cast to bf16, and replicated per
core as 4 row-banks of 25000 (int16 index range for the gather ucode).

Device pipeline per group of G=7 windows:
  - 4x dma_gather (one per bank, one SWDGE queue each) pull all 7
    windows' source rows for that bank in ONE instruction (~1664 idxs,
    zero-padded per window to its 128-multiple capacity), amortizing
    the ~1us per-instruction SWDGE fixed cost. single_packet=False is
    required: single-packet mode coalesces each DMA engine's stream
    into one packet capped at 16KB (= num_idxs/16 * 256B must fit).
  - ONE wide DVE tensor_tensor builds the whole group's 49 onehot
    chunks: oh[e, k, v] = (iota[v] == dstc[e, k]) in bf16, with
    stride-0 broadcast APs on both inputs.
  - Per window, TensorE accumulates agg^T[din, v] += X_chunk^T @ oh in
    PSUM (7 bf16 matmuls), then psB = outer(1/norm_dst, bias) +
    agg^T^T @ W (bias seeded via a K=1 matmul so the norm_dst scale
    applied next cancels on the bias term).
  - The scalar engine applies norm[dst] on the PSUM->SBUF copy into a
    per-group output tile; one batched DMA writes 7*128 output rows.

Host-side work is shard construction only: degree histograms + rsqrt
norms, node re-packing, edge bucketing/padding, constant tables, and
the inverse node permutation at unshard.
"""

import numpy as np

N_NODES = 100000
N_EDGES = 600000
D = 128
NC = 8            # cores
NPC = 12500       # nodes per core
P = 128           # partitions / window size
W = 98            # windows per core
NB = 4            # feat banks
BS = 25000        # bank size (int16-addressable)
CAP = 256         # max idx slots per (window, bank)
CPW = NB * 2      # chunks per window (uniform packing)
G = 7             # windows per gather group (must divide W)
SINGLE_PACKET = False  # True needs G*CAP/16*elem_bytes <= 16KB per engine
XG_BUFS = 4


W2 = 46           # windows with two light banks at packing level 2


def _lw(w, b, profile):
    """idx slots for (window, bank); light banks get 128, rotating so every
    bank is light equally often. profile (int): 0 uniform 256, 1 one light
    bank per window, 2 two light banks on the first W2 windows."""
    lv = int(profile)
    if lv >= 2 and w < W2:
        light = b == w % NB or b == (w + 1) % NB
    elif lv >= 1:
        light = b == w % NB
    else:
        light = False
    return 128 if light else 256


def _build_bass(
    rep=None,
    parts="all",
    bufs=None,
    g_win=None,
    single_packet=None,
    profile=True,
    out_pm=False,  # partition-major output benched 23% slower; keep strided
    invn_act=False,  # invn loads on Activation HWDGE benched 6% slower
    big_first=False,  # largest-bank-first gather issue benched no better
):
    """parts: 'all' | 'gather' | 'compute' | 'onehot' | 'matmul' isolates
    stages for benchmarking. rep: wrap the group loop in a hardware For_i.
    profile: rotated (256,256,256,128) per-window bank capacities (7 chunks
    per window) vs uniform 256 (8 chunks)."""
    import concourse.bacc as bacc
    import concourse.bass as bass
    import concourse.mybir as mybir
    import concourse.tile as tile

    f32 = mybir.dt.float32
    bf16 = mybir.dt.bfloat16
    i16 = mybir.dt.int16

    do_gather = parts in ("all", "gather")
    do_onehot = parts in ("all", "compute", "onehot")
    do_matmul = parts in ("all", "compute", "matmul")
    do_tail = parts in ("all", "compute")
    # For_i (rep) mode double-allocates pools for cross-iteration overlap;
    # use shallower buffers there so the bench build fits SBUF.
    XB = bufs or (3 if rep else XG_BUFS)
    OSB = 2 if rep else 3
    GW = g_win or G
    SP = SINGLE_PACKET if single_packet is None else single_packet
    NG = W // GW
    assert NG * GW == W
    PF = profile
    cpw = [sum(_lw(w, b, PF) for b in range(NB)) // 128 for w in range(W)]
    dcol = np.concatenate([[0], np.cumsum(cpw)]).astype(int)  # dstc col offs
    DSTC_COLS = int(dcol[-1])
    # idx col offsets, blocks ordered (bank, window)
    coloff = {}
    acc = 0
    for b in range(NB):
        for w in range(W):
            coloff[(b, w)] = acc
            acc += _lw(w, b, PF) // 16
    IDXC = acc

    nc = bacc.Bacc(
        None,
        target_bir_lowering=False,
        dynamic_dma_scratch_size=32768,
        num_swdge_queues=4,
    )
    feat_b = [
        nc.declare_dram_parameter(f"feat{b}", [BS, D], bf16, isOutput=False)
        for b in range(NB)
    ]
    w_d = nc.declare_dram_parameter("w", [D, D], bf16, isOutput=False)
    biasrow_d = nc.declare_dram_parameter("biasrow", [1, D], bf16, isOutput=False)
    invn_d = nc.declare_dram_parameter("invn", [1, W * P], bf16, isOutput=False)
    iota_d = nc.declare_dram_parameter("iota", [P, P], bf16, isOutput=False)
    idx_d = nc.declare_dram_parameter("idx", [P, IDXC], i16, isOutput=False)
    dstc_d = nc.declare_dram_parameter("dstc", [P, DSTC_COLS], bf16, isOutput=False)
    normd_d = nc.declare_dram_parameter("normd", [P, W], f32, isOutput=False)
    # partition-major: row p holds window-slot p's output for every window;
    # host unshard untransposes. Keeps each group's write one contiguous
    # 3.5KB-per-partition DMA (128 descs) vs 896 partition-strided 512B ones.
    out_shape = [P, W * D] if out_pm else [W * P, D]
    out_d = nc.declare_dram_parameter("out", out_shape, f32, isOutput=True)

    with tile.TileContext(nc) as tc:
        with (
            tc.tile_pool(name="const", bufs=1) as cp,
            tc.tile_pool(name="xg", bufs=XB) as xp,
            tc.tile_pool(name="oh", bufs=2) as ohp,
            tc.tile_pool(name="sb", bufs=8) as sbp,
            tc.tile_pool(name="osb", bufs=OSB) as obp,
            tc.tile_pool(name="iv", bufs=2) as ivp,
            tc.tile_pool(name="ps1", bufs=4, space="PSUM") as pp1,
            tc.tile_pool(name="ps2", bufs=4, space="PSUM") as pp2,
        ):
            idx_sb = cp.tile([P, IDXC], i16)
            nc.sync.dma_start(out=idx_sb[:], in_=idx_d[:])
            dstc_sb = cp.tile([P, DSTC_COLS], bf16)
            nc.sync.dma_start(out=dstc_sb[:], in_=dstc_d[:])
            normd_sb = cp.tile([P, W], f32)
            nc.sync.dma_start(out=normd_sb[:], in_=normd_d[:])
            iota_sb = cp.tile([P, P], bf16)
            nc.sync.dma_start(out=iota_sb[:], in_=iota_d[:])
            w_sb = cp.tile([D, D], bf16)
            nc.sync.dma_start(out=w_sb[:], in_=w_d[:])
            biasrow_sb = cp.tile([1, D], bf16)
            nc.sync.dma_start(out=biasrow_sb[:], in_=biasrow_d[:])

            import contextlib

            loop_cm = tc.For_i(0, rep, 1) if rep else contextlib.nullcontext()
            with loop_cm:
                for g in range(NG):
                    ws = list(range(g * GW, (g + 1) * GW))
                    nch_b = [
                        sum(_lw(w, b, PF) for w in ws) // 128 for b in range(NB)
                    ]
                    bkbase = np.concatenate([[0], np.cumsum(nch_b)]).astype(int)
                    totch = int(bkbase[-1])  # == GW * CPWW
                    xg = xp.tile([P, totch * D], bf16, tag="xg")
                    if not do_gather and do_matmul:
                        # benchmark mode: xg needs a writer off the
                        # critical DVE/PE path
                        nc.gpsimd.memset(xg[:], 0.0)
                    border = (
                        sorted(range(NB), key=lambda b: -nch_b[b])
                        if big_first
                        else range(NB)
                    )
                    for b in (border if do_gather else []):
                        cs = coloff[(b, g * GW)]
                        nidx = nch_b[b] * 128
                        nc.gpsimd.dma_gather(
                            out_ap=xg[
                                :, int(bkbase[b]) * D : int(bkbase[b + 1]) * D
                            ].rearrange("p (c r) -> p c r", r=D),
                            in_ap=feat_b[b][:, :],
                            idxs_ap=idx_sb[:, cs : cs + nidx // 16],
                            num_idxs=nidx,
                            num_idxs_reg=nidx,
                            elem_size=D,
                            single_packet=SP,
                            queue_num=b,
                        )
                    osb = obp.tile([P, GW * D], f32, tag="osb")
                    if do_tail:
                        # on the Activation HWDGE queue: SP program-orders
                        # this behind the previous group's output write,
                        # which only issues after that group's compute ends
                        invn_g = ivp.tile([1, GW * P], bf16, tag="invn")
                        (nc.scalar if invn_act else nc.sync).dma_start(
                            out=invn_g[:],
                            in_=invn_d[0:1, g * GW * P : (g + 1) * GW * P],
                        )
                    gch = int(dcol[(g + 1) * GW] - dcol[g * GW])  # == totch
                    ohg = ohp.tile([P, gch * P], bf16, tag="ohg")
                    if do_matmul and not do_onehot:
                        nc.gpsimd.memset(ohg[:], 0.0)
                    if do_onehot:
                        # all the group's onehot chunks in one wide DVE op:
                        # oh[e, k, v] = (iota[v] == dstc[e, k])
                        nc.vector.tensor_tensor(
                            out=ohg[:].rearrange("p (c v) -> p c v", v=P),
                            in0=iota_sb[:].unsqueeze(1).broadcast_to([P, gch, P]),
                            in1=dstc_sb[
                                :, int(dcol[g * GW]) : int(dcol[(g + 1) * GW])
                            ]
                            .unsqueeze(2)
                            .broadcast_to([P, gch, P]),
                            op=mybir.AluOpType.is_equal,
                        )
                    for wl in range(GW):
                        w_i = g * GW + wl
                        psA = pp1.tile([P, P], f32, tag="psA")
                        if do_matmul:
                            wbase = int(dcol[w_i] - dcol[g * GW])
                            cc = 0
                            for b in range(NB):
                                wch = (
                                    sum(_lw(w2, b, PF) for w2 in ws[:wl]) // 128
                                )
                                for j in range(_lw(w_i, b, PF) // 128):
                                    xoff = (int(bkbase[b]) + wch + j) * D
                                    ooff = (wbase + cc) * P
                                    nc.tensor.matmul(
                                        out=psA[:],
                                        lhsT=xg[:, xoff : xoff + D],
                                        rhs=ohg[:, ooff : ooff + P],
                                        start=(cc == 0),
                                        stop=(cc == cpw[w_i] - 1),
                                    )
                                    cc += 1
                        if not do_tail:
                            continue
                        aggT = sbp.tile([P, P], bf16, tag="aggT")
                        nc.scalar.activation(
                            aggT[:], psA[:], mybir.ActivationFunctionType.Copy
                        )
                        psB = pp2.tile([P, D], f32, tag="psB")
                        nc.tensor.matmul(
                            out=psB[:],
                            lhsT=invn_g[0:1, wl * P : (wl + 1) * P],
                            rhs=biasrow_sb[0:1, :],
                            start=True,
                            stop=False,
                        )
                        nc.tensor.matmul(
                            out=psB[:],
                            lhsT=aggT[:],
                            rhs=w_sb[:],
                            start=False,
                            stop=True,
                        )
                        nc.scalar.activation(
                            osb[:, wl * D : (wl + 1) * D],
                            psB[:],
                            mybir.ActivationFunctionType.Copy,
                            scale=normd_sb[:, w_i : w_i + 1],
                        )
                    if do_tail and out_pm:
                        nc.sync.dma_start(
                            out=out_d[:, g * GW * D : (g + 1) * GW * D],
                            in_=osb[:],
                        )
                    elif do_tail:
                        nc.sync.dma_start(
                            out=out_d[g * GW * P : (g + 1) * GW * P, :].rearrange(
                                "(c p) d -> p c d", p=P
                            ),
                            in_=osb[:].rearrange("p (c d) -> p c d", d=D),
                        )
    nc.compile()
    return nc


def _prep_shards(feat, weight, bias, src, dst, profile=2):
    import ml_dtypes

    bf16 = ml_dtypes.bfloat16
    feat = np.ascontiguousarray(np.asarray(feat, dtype=np.float32))
    weight = np.asarray(weight, dtype=np.float32)
    bias = np.asarray(bias, dtype=np.float32)
    src = np.asarray(src, dtype=np.int64)
    dst = np.asarray(dst, dtype=np.int64)

    deg = np.bincount(dst, minlength=N_NODES)
    norm = (1.0 / np.sqrt(np.maximum(deg, 1.0))).astype(np.float32)
    xs = (feat * norm[:, None]).astype(bf16)  # pre-scaled by norm[src]
    banks = [np.ascontiguousarray(xs[b * BS : (b + 1) * BS]) for b in range(NB)]
    bank = src // BS

    # per-node per-bank in-degree, for window packing
    d4 = np.zeros((N_NODES, NB), np.int64)
    for b in range(NB):
        np.add.at(d4[:, b], dst[bank == b], 1)

    # greedy re-pack of each core's nodes into W windows of <=128 nodes,
    # balancing per-bank edge loads against the per-window capacity targets
    targets = np.array(
        [[_lw(w, b, profile) for b in range(NB)] for w in range(W)], np.float64
    )
    slot_of = np.full(N_NODES, -1, np.int32)   # node -> slot (0..127)
    win_of = np.full(N_NODES, -1, np.int32)    # node -> window (0..97)
    perm = np.full((NC, W * P), -1, np.int64)  # (core, w*128+p) -> node
    for m in range(NC):
        nodes = np.arange(m * NPC, (m + 1) * NPC)
        dv = d4[nodes]
        order = np.argsort(-dv.sum(1), kind="stable")
        loads = np.zeros((W, NB), np.float64)
        counts = np.zeros(W, np.int32)
        for i in order:
            cand = ((loads + dv[i]) / targets).max(1)
            cand[counts >= P] = np.inf
            w = int(np.argmin(cand))
            n = nodes[i]
            win_of[n] = w
            slot_of[n] = counts[w]
            perm[m, w * P + counts[w]] = n
            loads[w] += dv[i]
            counts[w] += 1

    # bucket edges by (core, window, bank); position within bucket
    core_e = dst // NPC
    w_e = win_of[dst]
    key = (core_e * W + w_e) * NB + bank
    order = np.argsort(key, kind="stable")
    srcs, dsts, keys = src[order], dst[order], key[order]
    counts_e = np.bincount(keys, minlength=NC * W * NB)
    starts = np.zeros(NC * W * NB, np.int64)
    np.cumsum(counts_e[:-1], out=starts[1:])
    within = np.arange(N_EDGES, dtype=np.int64) - starts[keys]
    cnt3 = counts_e.reshape(NC, W, NB)
    if not (cnt3 <= targets[None]).all():
        assert profile, f"window/bank overflow: {cnt3.max()}"
        return _prep_shards(
            feat, weight, bias, src, dst, profile=int(profile) - 1
        )

    # slot-dense arrays [NC, W, NB, CAP]; pads gather row 0, masked by 255
    idx_full = np.zeros((NC, W, NB, CAP), np.int16)
    dstc_full = np.full((NC, W, NB, CAP), 255.0, np.float32)
    flat = ((keys * CAP) + within).astype(np.int64)
    idx_full.reshape(-1)[flat] = (srcs % BS).astype(np.int16)
    dstc_full.reshape(-1)[flat] = slot_of[dsts]

    # gather idx layout: L/16 cols per (w,b) block (value i at [i%16, i//16]),
    # blocks ordered (bank, window); replicate to 128 partitions
    idx16 = idx_full.reshape(NC, W, NB, 16, 16).transpose(0, 1, 2, 4, 3)
    blocks = [
        idx16[:, w, b, :, : _lw(w, b, profile) // 16]
        for b in range(NB)
        for w in range(W)
    ]
    idx_dev = np.ascontiguousarray(np.concatenate(blocks, axis=2))
    idx_dev = np.ascontiguousarray(np.tile(idx_dev, (1, 8, 1)))

    # onehot metadata [NC, 128, W*CPWW]: window-major, chunks (b asc, j asc)
    dcols = [
        dstc_full[:, w, b, : _lw(w, b, profile)].reshape(NC, -1, P)
        for w in range(W)
        for b in range(NB)
    ]
    dstc_dev = (
        np.concatenate(dcols, axis=1).transpose(0, 2, 1).astype(bf16)
    )

    norm_perm = np.where(perm >= 0, norm[np.maximum(perm, 0)], 0.0).astype(
        np.float32
    )
    normd = norm_perm.reshape(NC, W, P).transpose(0, 2, 1)  # [NC, 128, W]
    invn = (
        np.where(perm >= 0, 1.0 / np.maximum(norm_perm, 1e-30), 0.0)
        .astype(bf16)
        .reshape(NC, 1, W * P)
    )

    iota = np.ascontiguousarray(
        np.broadcast_to(np.arange(P, dtype=np.float32), (P, P)).astype(bf16)
    )
    biasrow = np.ascontiguousarray(bias.reshape(1, D).astype(bf16))
    wq = np.ascontiguousarray(weight.astype(bf16))

    in_maps = []
    for m in range(NC):
        im = {f"feat{b}": banks[b] for b in range(NB)}
        im.update(
            w=wq,
            biasrow=biasrow,
            invn=np.ascontiguousarray(invn[m]),
            iota=iota,
            idx=idx_dev[m],
            dstc=np.ascontiguousarray(dstc_dev[m]),
            normd=np.ascontiguousarray(normd[m]),
        )
        in_maps.append(im)
    return in_maps, perm, profile


def kernel(feat, weight, bias, src, dst):
    from concourse.bass_utils import run_bass_kernel_spmd

    in_maps, perm, prof = _prep_shards(feat, weight, bias, src, dst)
    nc = _build_bass(profile=prof)
    res = run_bass_kernel_spmd(nc, in_maps, list(range(NC)))
    out = np.empty((N_NODES, D), np.float32)
    for m in range(NC):
        o = res.results[m]["out"]
        mask = perm[m] >= 0
        out[perm[m][mask]] = o[mask]
    return out
